# revision 27
# baseline (speedup 1.0000x reference)
"""Trainium2 Bass kernel for nn_MoELayer_67619965108245 — fp8 DoubleRow version.

Dense MoE: B=64, N=55, D=512, E=8, L=4 SwiGLU layers per expert, H=2048.
Expert-parallel over 8 NeuronCores (one expert per core).

Same dT layout as the bf16 baseline (activations [d, t] in SBUF, tokens
n-major, N padded 55->56 so T=3584), but all SwiGLU matmuls run in
fp8e4m3 with perf_mode=DoubleRow (2 fp8 weights per PE cell, K=256 per
instruction). Quantization scales (powers of two, folded into existing
ops at zero cost):
  nt  = fp8(normed * 16)         Wg,Wv,Wo = fp8(W * 128)
  silu input descale 2^-11 via ACT scale; gv = fp8((vps*2^-6)*sil) and
  h += dps*2^-12 via DVE scalar_tensor_tensor.
Router runs in bf16 off a separate bf16 copy of x; softmax den/num and
the final d->1 projection use float32r (1 cyc/row at N=512).

Engine budget per 512-token chunk (steady state): PE ~96 DoubleRow MMs,
ACT 16 silu + 4 square + sqrt, DVE 16 gv + 4 nt + 4 h-add. The wo/hadd
phase of chunk c is emitted after gate/val of chunk c+1, and rmsnorm of
layer l+1 chunk c two chunks behind, so PE never waits on DVE/ACT lag.
"""

import numpy as np
import ml_dtypes

import concourse.bass as bass
import concourse.tile as tile
import concourse.mybir as mybir
from concourse.bass import ds, ts
from concourse.bass_utils import run_bass_kernel_spmd

B, N, D, E, L = 64, 55, 512, 8, 4
H = 4 * D
NP = 56          # padded node count
T = NP * B       # 3584 padded tokens, t = n*B + b
CH = 512         # token chunk (matmul free dim / PSUM bank)
NCH = T // CH    # 7
KD = D // 128    # 4 contraction chunks over d
KH = H // 128    # 16 contraction chunks over h
EPS = 1e-8

S_NT = 16.0
S_W = 128.0
S_GV = 32.0

fp32 = mybir.dt.float32
fp32r = mybir.dt.float32r
bf16 = mybir.dt.bfloat16
f8 = mybir.dt.float8e4
bf16_np = ml_dtypes.bfloat16
f8_np = ml_dtypes.float8_e4m3

DR = mybir.MatmulPerfMode.DoubleRow
MULT = mybir.AluOpType.mult
ADD = mybir.AluOpType.add

# Walrus in this toolchain rejects instructions carrying more than one
# semaphore wait; Tile's final drain aggregates many. Split extras onto
# preceding same-engine NOPs (identical sync semantics).
_MAX_WAITS = 1


def _split_excess_waits(nc, max_waits=_MAX_WAITS):
    for f in nc.m.functions:
        for bb in f.blocks:
            insts = bb.instructions
            i = 0
            while i < len(insts):
                inst = insts[i]
                si = inst.sync_info
                if si is None or si.on_wait is None or len(si.on_wait) <= max_waits:
                    i += 1
                    continue
                waits = list(si.on_wait)
                keep, extra = waits[-max_waits:], waits[:-max_waits]
                nops = []
                for j in range(0, len(extra), max_waits):
                    nops.append(
                        mybir.InstNoOp(
                            name=f"{inst.name}_ws{j}",
                            engine=inst.engine,
                            ins=[],
                            outs=[],
                            sync_info=mybir.SyncInfo(
                                on_wait=extra[j : j + max_waits], on_update=[]
                            ),
                        )
                    )
                inst.sync_info = mybir.SyncInfo(
                    on_wait=keep, on_update=list(si.on_update or [])
                )
                for k, nop in enumerate(nops):
                    insts.insert(i + k, nop)
                i += len(nops) + 1


def build_bass():
    nc = bass.Bass("TRN2", target_bir_lowering=False, debug=False, num_devices=E)

    xT_d = nc.dram_tensor("xT", [KD, 128, T], fp32, kind="ExternalInput").ap()
    xb_d = nc.dram_tensor("xb", [KD, 128, T], bf16, kind="ExternalInput").ap()
    wg_d = nc.dram_tensor("wg", [L, 128, KD, H], f8, kind="ExternalInput").ap()
    wv_d = nc.dram_tensor("wv", [L, 128, KD, H], f8, kind="ExternalInput").ap()
    wo_d = nc.dram_tensor("wo", [L, 128, KH, D], f8, kind="ExternalInput").ap()
    wr_d = nc.dram_tensor("wr", [128, NP, KD, E], bf16, kind="ExternalInput").ap()
    br_d = nc.dram_tensor("brt", [1, NP * E], bf16, kind="ExternalInput").ap()
    sel_d = nc.dram_tensor("sel", [E, 1], fp32r, kind="ExternalInput").ap()
    ones8_d = nc.dram_tensor("ones8", [128, 2, 16], f8, kind="ExternalInput").ap()
    onese_d = nc.dram_tensor("onese", [E, 1], fp32r, kind="ExternalInput").ap()
    wp_d = nc.dram_tensor("wp", [128, KD, 1], bf16, kind="ExternalInput").ap()
    bp_d = nc.dram_tensor("bps", [1, 1], fp32, kind="ExternalInput").ap()
    u_d = nc.dram_tensor("u", [1, T], fp32, kind="ExternalOutput").ap()

    with tile.TileContext(nc) as tc:
        from contextlib import ExitStack

        with ExitStack() as ctx:
            const = ctx.enter_context(tc.tile_pool(name="const", bufs=1))
            hp = ctx.enter_context(tc.tile_pool(name="hpool", bufs=1))
            xbp = ctx.enter_context(tc.tile_pool(name="xbp", bufs=3))
            wpg = ctx.enter_context(tc.tile_pool(name="wpg", bufs=2))
            wpv = ctx.enter_context(tc.tile_pool(name="wpv", bufs=2))
            wpo = ctx.enter_context(tc.tile_pool(name="wpo", bufs=2))
            nrm = ctx.enter_context(tc.tile_pool(name="nrm", bufs=10))
            sqp = ctx.enter_context(tc.tile_pool(name="sqp", bufs=2))
            gvp = ctx.enter_context(tc.tile_pool(name="gvp", bufs=2))
            silup = ctx.enter_context(tc.tile_pool(name="silup", bufs=4))
            smallp = ctx.enter_context(tc.tile_pool(name="smallp", bufs=2))
            routp = ctx.enter_context(tc.tile_pool(name="routp", bufs=2))
            outp = ctx.enter_context(tc.tile_pool(name="outp", bufs=2))
            hbp = ctx.enter_context(tc.tile_pool(name="hbp", bufs=2))
            pg = ctx.enter_context(tc.tile_pool(name="pg", bufs=2, space="PSUM"))
            pv = ctx.enter_context(tc.tile_pool(name="pv", bufs=2, space="PSUM"))
            pd = ctx.enter_context(tc.tile_pool(name="pd", bufs=2, space="PSUM"))
            pm = ctx.enter_context(tc.tile_pool(name="pm", bufs=2, space="PSUM"))

            # ---- constants (DMAs emitted below in critical-path order) ----
            ones8 = const.tile([128, 2, 16], f8, name="ones8")
            ones_m_bf = const.tile([1, 128], bf16, name="ones_m_bf")
            nc.vector.memset(ones_m_bf, 1.0)
            ones_b_bf = const.tile([1, B], bf16, name="ones_b_bf")
            nc.vector.memset(ones_b_bf, 1.0)
            ones_e_f = const.tile([E, 1], fp32r, name="ones_e_f")
            eps_sb = const.tile([1, 1], fp32, name="eps_sb")
            nc.vector.memset(eps_sb, EPS)
            sel_sb = const.tile([E, 1], fp32r, name="sel_sb")
            br_sb = const.tile([1, NP * E], bf16, name="br_sb")
            wr_sb = const.tile([128, NP, KD, E], bf16, name="wr_sb")
            wp_sb = const.tile([128, KD, 1], bf16, name="wp_sb")
            bp_sb = const.tile([1, 1], fp32, name="bp_sb")
            w_sb = const.tile([1, T], fp32, name="w_sb")  # router weight row

            # ---- residual state (fp32, dT layout) ----
            # Chunked DMAs, router xb first: the router only waits for
            # chunk 0 of xb, not the whole 11MB of activations.
            h = [hp.tile([128, T], fp32, name=f"h{k}", tag=f"h{k}") for k in range(KD)]
            xbt = {}

            def load_xb(c):
                cs = ds(c * CH, CH)
                xc = xbp.tile([128, KD, CH], bf16, name=f"xb{c}", tag="xb")
                for k in range(KD):
                    nc.sync.dma_start(xc[:, k, :], xb_d[k][:, cs])
                xbt[c] = xc

            def load_h(c):
                cs = ds(c * CH, CH)
                for k in range(KD):
                    nc.sync.dma_start(h[k][:, cs], xT_d[k][:, cs])

            # router-critical DMAs first, tail-only constants last
            nc.sync.dma_start(wr_sb[:], wr_d[:])
            load_xb(0)
            nc.sync.dma_start(br_sb[:], br_d[:])
            nc.sync.dma_start(sel_sb[:], sel_d[:])
            nc.sync.dma_start(ones_e_f[:], onese_d[:])
            load_xb(1)
            load_h(0)
            nc.sync.dma_start(ones8[:], ones8_d[:])
            nc.sync.dma_start(wp_sb[:], wp_d[:])
            nc.sync.dma_start(bp_sb[:], bp_d[:])

            # ---- router: all-E logits, softmax, own-expert weight row ----
            NPC = CH // B  # nodes per chunk

            def router(c):
                cs = ds(c * CH, CH)
                xc = xbt.pop(c)
                lg = pm.tile([128, CH], fp32, name=f"lg{c}", tag="pm")
                for ni in range(NPC):
                    n = c * NPC + ni
                    off = ni * B
                    for k in range(KD):
                        nc.tensor.matmul(
                            lg[0:E, ds(off, B)],
                            wr_sb[:, n, k, :],
                            xc[:, k, ds(off, B)],
                            start=(k == 0),
                            stop=False,
                        )
                    nc.tensor.matmul(
                        lg[0:E, ds(off, B)],
                        br_sb[0:1, ds(n * E, E)],
                        ones_b_bf[:],
                        start=False,
                        stop=True,
                    )
                expc = routp.tile([E, CH], fp32r, name=f"expc{c}", tag="expc")
                nc.scalar.activation(
                    expc[:], lg[0:E, :], mybir.ActivationFunctionType.Exp
                )
                den = pm.tile([128, CH], fp32, name=f"den{c}", tag="pm")
                nc.tensor.matmul(
                    den[0:1, :], ones_e_f[:], expc[:], start=True, stop=True
                )
                num = pd.tile([128, CH], fp32, name=f"num{c}", tag="pd")
                nc.tensor.matmul(
                    num[0:1, :], sel_sb[:], expc[:], start=True, stop=True
                )
                rden = smallp.tile([1, CH], fp32, name=f"rden{c}", tag="rden")
                nc.vector.reciprocal(rden[:], den[0:1, :])
                nc.vector.tensor_mul(w_sb[:, cs], num[0:1, :], rden[:])

            # ---- per-layer weight tiles (split DMAs so the first j-tiles
            # arrive before the full tensor) ----
            def load_weights(l):
                wg_sb = wpg.tile([128, KD, H], f8, name=f"wg{l}", tag="wg")
                wv_sb = wpv.tile([128, KD, H], f8, name=f"wv{l}", tag="wv")
                wo_sb = wpo.tile([128, KH, D], f8, name=f"wo{l}", tag="wo")
                for s in range(2):
                    hs = ds(s * (H // 2), H // 2)
                    nc.sync.dma_start(wg_sb[:, :, hs], wg_d[l][:, :, hs])
                for s in range(2):
                    hs = ds(s * (H // 2), H // 2)
                    nc.sync.dma_start(wv_sb[:, :, hs], wv_d[l][:, :, hs])
                for s in range(2):
                    dsl = ds(s * (D // 2), D // 2)
                    nc.sync.dma_start(wo_sb[:, :, dsl], wo_d[l][:, :, dsl])
                return wg_sb, wv_sb, wo_sb

            normed = {}  # (l, c) -> nt tile

            def norm(l, c):
                cs = ds(c * CH, CH)
                sq = sqp.tile([128, KD, CH], f8, name=f"sq{l}_{c}", tag="sq")
                for k in range(KD):
                    nc.scalar.activation(
                        sq[:, k, :], h[k][:, cs],
                        mybir.ActivationFunctionType.Square,
                    )
                msq = pm.tile([128, CH], fp32, name=f"ms{l}_{c}", tag="pm")
                for kk in range(KD // 2):
                    nc.tensor.matmul(
                        msq[0:1, :],
                        ones8[:, :, 0:1],
                        sq[:, ds(2 * kk, 2), :],
                        start=(kk == 0),
                        stop=(kk == KD // 2 - 1),
                        perf_mode=DR,
                    )
                std = smallp.tile([1, CH], fp32, name=f"std{l}_{c}", tag="std")
                nc.scalar.activation(
                    std[:],
                    msq[0:1, :],
                    mybir.ActivationFunctionType.Sqrt,
                    bias=eps_sb[:],
                    scale=1.0 / D,
                )
                rstd = smallp.tile([1, CH], bf16, name=f"rstd{l}_{c}", tag="rstd")
                with nc.allow_low_precision(
                    reason="rstd feeds fp8 normed; bf16 rstd is free precision-wise"
                ):
                    nc.vector.reciprocal(rstd[:], std[:])
                bc = pm.tile([128, CH], fp32, name=f"bc{l}_{c}", tag="pm")
                nc.tensor.matmul(bc[:], ones_m_bf[:], rstd[:], start=True, stop=True)
                nt = nrm.tile([128, KD, CH], f8, name=f"nt{l}_{c}", tag="nt")
                for k in range(KD):
                    nc.vector.scalar_tensor_tensor(
                        nt[:, k, :], h[k][:, cs], S_NT, bc[:], MULT, MULT
                    )
                normed[(l, c)] = nt

            gvs = {}  # c -> gv tile of current layer

            def gateval(l, c, wg_sb, wv_sb):
                nt = normed.pop((l, c))
                gv = gvp.tile([128, KH, CH], f8, name=f"gv{l}_{c}", tag="gv")
                for j in range(KH):
                    gps = pg.tile([128, CH], fp32, name=f"g{l}_{c}_{j}", tag="pg")
                    vps = pv.tile([128, CH], fp32, name=f"v{l}_{c}_{j}", tag="pv")
                    for kk in range(KD // 2):
                        nc.tensor.matmul(
                            gps[:],
                            wg_sb[:, ds(2 * kk, 2), ts(j, 128)],
                            nt[:, ds(2 * kk, 2), :],
                            start=(kk == 0),
                            stop=(kk == KD // 2 - 1),
                            perf_mode=DR,
                        )
                    for kk in range(KD // 2):
                        nc.tensor.matmul(
                            vps[:],
                            wv_sb[:, ds(2 * kk, 2), ts(j, 128)],
                            nt[:, ds(2 * kk, 2), :],
                            start=(kk == 0),
                            stop=(kk == KD // 2 - 1),
                            perf_mode=DR,
                        )
                    sil = silup.tile([128, CH], f8, name=f"sl{l}_{c}_{j}", tag="sil")
                    nc.scalar.activation(
                        sil[:], gps[:],
                        mybir.ActivationFunctionType.Silu,
                        scale=1.0 / (S_NT * S_W),
                    )
                    nc.vector.scalar_tensor_tensor(
                        gv[:, j, :], vps[:], S_GV / (S_NT * S_W), sil[:], MULT, MULT
                    )
                gvs[c] = gv

            def wo_hadd(l, c, wo_sb):
                cs = ds(c * CH, CH)
                gv = gvs.pop(c)
                for i in range(KD):
                    dps = pd.tile([128, CH], fp32, name=f"d{l}_{c}_{i}", tag="pd")
                    for jj in range(KH // 2):
                        nc.tensor.matmul(
                            dps[:],
                            wo_sb[:, ds(2 * jj, 2), ts(i, 128)],
                            gv[:, ds(2 * jj, 2), :],
                            start=(jj == 0),
                            stop=(jj == KH // 2 - 1),
                            perf_mode=DR,
                        )
                    nc.vector.scalar_tensor_tensor(
                        h[i][:, cs], dps[:], 1.0 / (S_GV * S_W), h[i][:, cs],
                        MULT, ADD,
                    )

            def final(c):
                cs = ds(c * CH, CH)
                hb = hbp.tile([128, KD, CH], bf16, name=f"hb{c}", tag="hb")
                for k in range(KD):
                    nc.scalar.activation(
                        hb[:, k, :], h[k][:, cs],
                        mybir.ActivationFunctionType.Copy,
                    )
                eo = pm.tile([128, CH], fp32, name=f"eo{c}", tag="pm")
                for k in range(KD):
                    nc.tensor.matmul(
                        eo[0:1, :],
                        wp_sb[:, k, :],
                        hb[:, k, :],
                        start=(k == 0),
                        stop=(k == KD - 1),
                    )
                eos = outp.tile([1, CH], fp32, name=f"eos{c}", tag="eos")
                nc.scalar.activation(
                    eos[:], eo[0:1, :],
                    mybir.ActivationFunctionType.Identity,
                    bias=bp_sb[:],
                )
                us = outp.tile([1, CH], fp32, name=f"us{c}", tag="us")
                nc.vector.tensor_mul(us[:], eos[:], w_sb[:, cs])
                nc.sync.dma_start(u_d[0:1, cs], us[:])

            # ---- startup: router + layer-0 rmsnorm, chunked-JIT DMAs ----
            weights = load_weights(0)
            for c in range(NCH):
                if c + 2 < NCH:
                    load_xb(c + 2)
                if c + 1 < NCH:
                    load_h(c + 1)
                router(c)
                norm(0, c)

            # ---- layers, software-pipelined ----
            for l in range(L):
                wg_sb, wv_sb, wo_sb = weights
                for c in range(NCH):
                    gateval(l, c, wg_sb, wv_sb)
                    if c == 2 and l < L - 1:
                        weights = load_weights(l + 1)
                    if c >= 1:
                        wo_hadd(l, c - 1, wo_sb)
                        if l < L - 1 and c >= 2:
                            norm(l + 1, c - 2)
                        if l == L - 1 and c >= 2:
                            final(c - 2)
                wo_hadd(l, NCH - 1, wo_sb)
                if l < L - 1:
                    norm(l + 1, NCH - 2)
                    norm(l + 1, NCH - 1)
                else:
                    final(NCH - 2)
                    final(NCH - 1)

    _split_excess_waits(nc)
    return nc


_CACHE = {}


def _get_nc():
    if "nc" not in _CACHE:
        _CACHE["nc"] = build_bass()
    return _CACHE["nc"]


def _prep_inputs(x, scale, Wg, Wv, Wo, Wp, bp, Wr, br):
    x = np.asarray(x, np.float32)
    scale = np.asarray(scale, np.float32)
    Wg = np.asarray(Wg, np.float32)
    Wv = np.asarray(Wv, np.float32)
    Wo = np.asarray(Wo, np.float32)
    Wp = np.asarray(Wp, np.float32)
    bp = np.asarray(bp, np.float32)
    Wr = np.asarray(Wr, np.float32)
    br = np.asarray(br, np.float32)

    # xT: [d, n, b] padded -> [KD, 128, T]; bf16 copy for the router
    xt = np.zeros((D, NP, B), np.float32)
    xt[:, :N, :] = x.transpose(2, 1, 0)
    xT = np.ascontiguousarray(xt.reshape(KD, 128, T))
    xTb = xT.astype(bf16_np)

    # router tensors (shared by all cores)
    wr_full = np.zeros((NP, E, D), np.float32)
    wr_full[:N] = Wr
    wr_prep = np.ascontiguousarray(
        wr_full.transpose(2, 0, 1).reshape(KD, 128, NP, E).transpose(1, 2, 0, 3)
    ).astype(bf16_np)
    br_full = np.zeros((NP, E), np.float32)
    br_full[:N] = br
    br_prep = np.ascontiguousarray(br_full.reshape(1, NP * E)).astype(bf16_np)

    # fold RMSNorm scale into Wg/Wv rows, apply fp8 pre-scales: (L, E, D, H)
    # clip to TRN e4m3 range (|x| >= 256 encodes as inf on TRN)
    wg_eff = np.clip(Wg * scale[:, :, :, None] * S_W, -224.0, 224.0)
    wv_eff = np.clip(Wv * scale[:, :, :, None] * S_W, -224.0, 224.0)
    wo_eff = np.clip(Wo * S_W, -224.0, 224.0)

    in_maps = []
    for e in range(E):
        wg_p = np.ascontiguousarray(
            wg_eff[:, e].reshape(L, KD, 128, H).transpose(0, 2, 1, 3)
        ).astype(f8_np)
        wv_p = np.ascontiguousarray(
            wv_eff[:, e].reshape(L, KD, 128, H).transpose(0, 2, 1, 3)
        ).astype(f8_np)
        wo_p = np.ascontiguousarray(
            wo_eff[:, e].reshape(L, KH, 128, D).transpose(0, 2, 1, 3)
        ).astype(f8_np)
        wp_p = np.ascontiguousarray(
            Wp[e].reshape(KD, 128, 1).transpose(1, 0, 2)
        ).astype(bf16_np)
        sel = np.zeros((E, 1), np.float32)
        sel[e, 0] = 1.0
        in_maps.append(
            {
                "xT": xT,
                "xb": xTb,
                "wg": wg_p,
                "wv": wv_p,
                "wo": wo_p,
                "wr": wr_prep,
                "brt": br_prep,
                "sel": sel,
                "ones8": np.ones((128, 2, 16), f8_np),
                "onese": np.ones((E, 1), np.float32),
                "wp": wp_p,
                "bps": np.array([[bp[e]]], np.float32),
            }
        )
    return in_maps


def _combine(results):
    u = np.zeros(T, np.float64)
    for r in results:
        u += r["u"].reshape(T).astype(np.float64)
    return np.ascontiguousarray(u.reshape(NP, B)[:N, :].T).astype(np.float32)


def kernel(x, scale, Wg, Wv, Wo, Wp, bp, Wr, br):
    nc = _get_nc()
    in_maps = _prep_inputs(x, scale, Wg, Wv, Wo, Wp, bp, Wr, br)
    res = run_bass_kernel_spmd(nc, in_maps, list(range(E)))
    return _combine(res.results)
